# revision 4
# baseline (speedup 1.0000x reference)
"""Trainium2 Bass kernel for nn_DennisNode (T=1024, N=65536, 8 cores).

Recurrence structure (per node; health/phase are dead code):
    tension = |zn_t - Z|
    E'  = min(max(0.98 E + 100 tension, 0), 1e6)
    X1' = (X1 + if + 0.005 E')(1 - cp)
    phi = X1' - ir/2;  Z' = (1-dm)phi * dm|phi| + (ten<0.01 ? -cb : cb)*0.1 + na*nz_t

E saturates at 1e6 on every node by t=146 (verified on host from device
state), X1 then contracts (ratio 1-cp) to a global fp32 fixed point, reached
exactly by t=191. Three phases, ONE bass program:
  phase 1 (t=0..146):   exact serial chain, 6 custom-DVE ops/step (each op
                        is a multi-stage fused instruction whose per-stage
                        fp32 rounding reproduces the reference op order).
  phase 2 (t=147..191): saturated chain, 2 ops/step (x1b state stays exact;
                        Z uses a quadratic approximation, error ~1e-6 rel).
  phase 3 (t=192..1023): Z = na*nz + cc_t with cc_t from the input-independent
                        scalar attractor orbit (host-computed).  Runs on the
                        otherwise-idle ACT engine + DMA, CONCURRENT with
                        phases 1-2 on the vector engine.
Host verifies the absorbing state from device state_out (exact conditions);
falls back to an exact numpy simulation if any check fails.
"""
import sys

sys.path.insert(0, "/opt/trn_rl_repo")

import numpy as np

import concourse.bass as bass
import concourse.mybir as mybir
from concourse.tile import TileContext
from concourse.bass_utils import run_bass_kernel_spmd
from concourse.dve_spec import (
    Spec, Src0, Src1, C0, C1, C2, Zero, One, maxx, minn,
    lower as _dve_lower, _has_src1,
)
from concourse.dve_ops import DveOp, OPS, CUSTOM_DVE_SPECS, _SUB_OPCODE_FOR_NAME
from concourse.dve_uop import DveOpSpec

F32 = mybir.dt.float32
Alu = mybir.AluOpType
f32 = np.float32

T, N = 1024, 65536
NCORES = 8
NL = N // NCORES          # nodes per core (8192)
P = 128                   # partitions
FDN = NL // P             # free-dim nodes per core (64)
TC1 = 147                 # phase-1 steps (all nodes E-saturated after)
TC2 = 192                 # phase-2 end / phase-3 start
TTAIL = T - TC2           # 832
TB = 48                   # steps per DMA block in phases 1+2 (4 blocks)
SAT_E = f32(1e6)

# ---------------------------------------------------------------- walrus fix
_ctr = [0]


def _fix_sync_waits(nc, max_waits: int = 1):
    """This walrus build rejects >1 semaphore wait per instruction
    (CoreV3 setupSyncWait). Hoist excess waits onto same-engine NOPs."""
    for fn in nc.m.functions:
        for blk in fn.blocks:
            out, changed = [], False
            for ins in blk.instructions:
                si = ins.sync_info
                if si is not None and len(si.on_wait) > max_waits:
                    waits = list(si.on_wait)
                    head, tail = waits[:-max_waits], waits[-max_waits:]
                    for j in range(0, len(head), max_waits):
                        _ctr[0] += 1
                        nop = mybir.InstNoOp(
                            name=f"I-waitsplit-{_ctr[0]}",
                            engine=ins.engine,
                            bass_nofuse=True,
                            sync_info=mybir.SyncInfo(
                                on_wait=head[j : j + max_waits], on_update=[]
                            ),
                        )
                        nc.register_instruction(nop, overwrite=True)
                        out.append(nop)
                    ins.sync_info = mybir.SyncInfo(
                        on_wait=tail, on_update=list(si.on_update)
                    )
                    changed = True
                out.append(ins)
            if changed:
                blk.instructions = out


# ------------------------------------------------------------ custom DVE ops
def _register_op(name, spec):
    for op in OPS:
        if op.name == name:
            return op
    row = max(_SUB_OPCODE_FOR_NAME.values()) + 1
    assert row < 0x20, "out of custom-DVE opcode rows"
    _SUB_OPCODE_FOR_NAME[name] = row
    shas = {}
    for ver in ("v3", "v4"):
        try:
            uops = _dve_lower(spec, ver=ver)
            shas[ver] = DveOpSpec(
                name=name, opcode=row, uops=uops, rd1_en=_has_src1(spec)
            ).sha(ver)
        except Exception:
            if ver == "v3":
                raise
    op = DveOp(name, spec, subdim=False, uops_sha=shas)
    OPS.append(op)
    CUSTOM_DVE_SPECS[name] = spec
    return op


def _mk_ops():
    ops = {}
    # DN_TEN: t2q = fl(25*|zn - Z|)   [STT: in1=Zprev elementwise; s0=25]
    _d = Src0 - Src1
    ops["ten"] = _register_op("DN_TEN_ANT", Spec(
        body=maxx(_d, Zero - _d) * C0,
        reference=lambda in0, in1, s0, s1, imm2: (
            lambda d: f32(np.maximum(d, f32(-d)) * f32(s0))
        )(f32(np.asarray(in0, f32) - in1)),
    ))
    # DN_EN: G' = min(fl(fl(s0*G) + t2q), s1)   [STT: in1=t2q; G = E/4]
    ops["en"] = _register_op("DN_EN_ANT", Spec(
        body=minn(Src0 * C0 + Src1, C1),
        reference=lambda in0, in1, s0, s1, imm2: np.minimum(
            f32(f32(np.asarray(in0, f32) * f32(s0)) + in1), f32(s1)
        ),
    ))
    # DN_X1: x1b' = fl(fl(X1'+s1) + fl(s1*G)); X1' = fl(s0*x1b)+x1b
    #   [STT: in0=x1b, in1=G; s0=-cp, s1=if; REQUIRES fl(if) == 4*fl(0.005)]
    _x1p = Src0 * C0 + Src0
    ops["x1"] = _register_op("DN_X1_ANT", Spec(
        body=(_x1p + C1) + Src1 * C1,
        reference=lambda in0, in1, s0, s1, imm2: (
            lambda x1p: f32(f32(x1p + f32(s1)) + f32(np.asarray(in1, f32) * f32(s1)))
        )(f32(f32(np.asarray(in0, f32) * f32(s0)) + in0)),
    ))
    # DN_P: q = fl(sd*(S-sd)); S = fl(fl(X1'-s1)+X1'); sd = fl(S*imm2);
    #   X1' = fl(s0*x1b)+x1b   [TTSS: in0=x1b; s0=-cp, s1=ir, imm2=dm1]
    _x1n = Src0 * C0 + Src0
    _S = (_x1n - C1) + _x1n
    _sd = _S * C2
    ops["p"] = _register_op("DN_P_ANT", Spec(
        body=_sd * (_S - _sd),
        reference=lambda in0, in1, s0, s1, imm2: (
            lambda S: (lambda sd: f32(sd * f32(S - sd)))(f32(S * f32(imm2)))
        )((lambda x: f32(f32(x - f32(s1)) + x))(
            f32(f32(np.asarray(in0, f32) * f32(s0)) + in0))),
    ))
    # DN_CC4: cc4 = fl((2b-1)*s1 + q), b = (t2q >= s0)
    #   [STT: in0=q, in1=t2q; s0=0.25, s1=4*c01]
    _b = Src1 >= C0
    ops["cc4"] = _register_op("DN_CC4_ANT", Spec(
        body=((_b + _b) - One) * C1 + Src0,
        reference=lambda in0, in1, s0, s1, imm2: (
            lambda b: f32(f32(f32(f32(b + b) - f32(1.0)) * f32(s1)) + in0)
        )((np.asarray(in1, f32) >= f32(s0)).astype(f32)),
    ))
    # DN_Z: Z = fl(fl(s0*nz) + fl(s1*cc4))   [STT: in0=nz, in1=cc4;
    #   s0=na, s1=0.25]
    ops["z"] = _register_op("DN_Z_ANT", Spec(
        body=Src0 * C0 + Src1 * C1,
        reference=lambda in0, in1, s0, s1, imm2: f32(
            f32(np.asarray(in0, f32) * f32(s0)) + f32(np.asarray(in1, f32) * f32(s1))
        ),
    ))
    # DN_X1SAT: x1b' = fl(fl(X1'+s1) + imm2)   [TTSS: in0=x1b;
    #   s0=-cp, s1=if, imm2=aa_sat]
    _x1ps = Src0 * C0 + Src0
    ops["x1sat"] = _register_op("DN_X1SAT_ANT", Spec(
        body=(_x1ps + C1) + C2,
        reference=lambda in0, in1, s0, s1, imm2: (
            lambda x1p: f32(f32(x1p + f32(s1)) + f32(imm2))
        )(f32(f32(np.asarray(in0, f32) * f32(s0)) + in0)),
    ))
    # DN_PZSAT: Z ~= fl(fl(fl((x1b-s0)^2)*s1) + w)   [STT: in0=x1b, in1=w]
    _u = Src0 - C0
    ops["pzsat"] = _register_op("DN_PZSAT_ANT", Spec(
        body=(_u * _u) * C1 + Src1,
        reference=lambda in0, in1, s0, s1, imm2: (
            lambda u: f32(f32(f32(u * u) * f32(s1)) + in1)
        )(f32(np.asarray(in0, f32) - f32(s0))),
    ))
    return ops


_DN_OPS = None


def _dn_ops():
    global _DN_OPS
    if _DN_OPS is None:
        _DN_OPS = _mk_ops()
    return _DN_OPS


# ---------------------------------------------------------------- constants
def _consts(scal):
    cb, iff, ir, cp, dm, na = (f32(scal[k]) for k in (
        "coupling_base", "internal_forward", "internal_reverse",
        "center_pull", "damping", "noise_amplitude"))
    c = {}
    c["iff"], c["ir"], c["cp"], c["dm"], c["na"] = iff, ir, cp, dm, na
    c["ncp"] = f32(-cp)
    c["c01"] = f32(cb * f32(0.1))
    c["c01x4"] = f32(4.0) * f32(cb * f32(0.1))
    c["dm1"] = f32(f32(1.0) - dm)
    c["aa_sat"] = f32(f32(0.005) * SAT_E)
    c["g0"] = f32(f32(0.01) / f32(4.0))
    # phase-2 quadratic approximation coefficients (fp64 -> fp32)
    c["cA"] = f32(float(ir) / (2.0 * (1.0 - float(cp))))
    c["cB"] = f32(float(dm) * float(c["dm1"]) * (1.0 - float(cp)) ** 2)
    # specialization for the 6-op phase-1 step
    c["x1_fused_ok"] = bool(iff == f32(4.0) * f32(0.005))
    return c


# ------------------------------------------------------------- host orbit
def _x1_map(x, c):
    x1b = f32(f32(x + c["iff"]) + c["aa_sat"])
    return f32(f32(x1b * c["ncp"]) + x1b), x1b


def _attractor(c, iters=600):
    x = f32(0.0)
    for _ in range(iters):
        x, _ = _x1_map(x, c)
    return x


def _cc_orbit(c, x_start, nsteps):
    """cc_t for t = TC2..: exact fp32 orbit from x_start (= X1 at t=191)."""
    cc = np.empty(nsteps, f32)
    x = x_start
    for i in range(nsteps):
        x, _ = _x1_map(x, c)
        s = f32(f32(x - c["ir"]) + x)
        sd = f32(s * c["dm1"])
        q = f32(sd * f32(s - sd))
        cc[i] = f32(f32(q * f32(0.25)) + c["c01"])
    return cc


# ---------------------------------------------------------------- main build
def _build_main(scal, reps=1):
    c = _consts(scal)
    assert c["x1_fused_ok"], "phase-1 6-op step requires fl(if) == 4*fl(0.005)"
    ops = _dn_ops()
    V = None  # set below

    nc = bass.Bass()
    znh = nc.dram_tensor("znh", [P, TC1, FDN], F32, kind="ExternalInput")
    nzh = nc.dram_tensor("nzh", [P, TC2, FDN], F32, kind="ExternalInput")
    nzt = nc.dram_tensor("nzt", [TTAIL, NL], F32, kind="ExternalInput")
    ccv = nc.dram_tensor("cc", [TTAIL, 1], F32, kind="ExternalInput")
    zoh = nc.dram_tensor("zout_h", [P, TC2, FDN], F32, kind="ExternalOutput")
    zot = nc.dram_tensor("zout_t", [TTAIL, NL], F32, kind="ExternalOutput")
    sto = nc.dram_tensor("state_out", [2, P, FDN], F32, kind="ExternalOutput")

    V = nc.vector
    cdve = V._custom_dve
    Ident = mybir.ActivationFunctionType.Identity
    ntile = (TTAIL + P - 1) // P

    with TileContext(nc) as tc:
        with (
            tc.tile_pool(name="p3", bufs=3) as p3p,
            tc.tile_pool(name="ccp", bufs=2) as ccp,
            tc.tile_pool(name="st", bufs=1) as stp,
            tc.tile_pool(name="io", bufs=2) as iop,
            tc.tile_pool(name="wk", bufs=2) as wkp,
        ):
            def body(tag=""):
                # ---------- phase 3: ACT engine + DMA, input-independent ----
                for b in range(ntile):
                    pp = P if (b + 1) * P <= TTAIL else TTAIL - b * P
                    t0 = b * P
                    tl = p3p.tile([P, NL], F32, name=f"p3_{b}{tag}", tag="p3")
                    cct = ccp.tile([P, 1], F32, name=f"cct{b}{tag}", tag="cct")
                    nc.sync.dma_start(out=cct[:pp], in_=ccv[t0:t0 + pp])
                    nc.sync.dma_start(out=tl[:pp], in_=nzt[t0:t0 + pp])
                    nc.scalar.activation(out=tl[:pp], in_=tl[:pp], func=Ident,
                                         bias=cct[:pp], scale=float(c["na"]))
                    nc.sync.dma_start(out=zot[t0:t0 + pp], in_=tl[:pp])

                # ---------- phases 1+2: serial chain on DVE ----------------
                Zs = stp.tile([P, FDN], F32, name=f"Zs{tag}")
                x1b = stp.tile([P, FDN], F32, name=f"x1b{tag}")
                G = stp.tile([P, FDN], F32, name=f"G{tag}")
                V.memset(Zs[:], 0.0)
                V.memset(x1b[:], 0.0)
                V.memset(G[:], float(c["g0"]))

                zprev = Zs[:]
                nblk = (TC2 + TB - 1) // TB
                for b in range(nblk):
                    t0, t1 = b * TB, min((b + 1) * TB, TC2)
                    n1 = max(0, min(t1, TC1) - t0)       # phase-1 rows
                    n2 = (t1 - t0) - n1                  # phase-2 rows
                    nzt_ = iop.tile([P, t1 - t0, FDN], F32,
                                    name=f"nzb{b}{tag}", tag="nzb")
                    nc.sync.dma_start(out=nzt_[:], in_=nzh[:, t0:t1])
                    if n1 > 0:
                        znt = iop.tile([P, n1, FDN], F32,
                                       name=f"znb{b}{tag}", tag="znb")
                        nc.sync.dma_start(out=znt[:], in_=znh[:, t0:t0 + n1])
                    if n2 > 0:
                        # w = fl(fl(na*nz) + c01) for the phase-2 rows
                        wv = nzt_[:, n1:]
                        V.tensor_scalar(out=wv, in0=wv,
                                        scalar1=float(c["na"]),
                                        scalar2=float(c["c01"]),
                                        op0=Alu.mult, op1=Alu.add)
                    for s in range(t1 - t0):
                        t = t0 + s
                        nz2 = nzt_[:, s]
                        if t < TC1:
                            zn2 = znt[:, s]
                            t2q = wkp.tile([P, FDN], F32,
                                           name=f"t2q_{t}{tag}", tag="t2q")
                            q = wkp.tile([P, FDN], F32,
                                         name=f"q_{t}{tag}", tag="q")
                            cc4 = wkp.tile([P, FDN], F32,
                                           name=f"cc4_{t}{tag}", tag="cc4")
                            cdve(ops["ten"], out=t2q[:], in0=zn2,
                                 in1=zprev, s0=25.0)
                            cdve(ops["en"], out=G[:], in0=G[:], in1=t2q[:],
                                 s0=0.98, s1=250000.0)
                            cdve(ops["x1"], out=x1b[:], in0=x1b[:], in1=G[:],
                                 s0=float(c["ncp"]), s1=float(c["iff"]))
                            cdve(ops["p"], out=q[:], in0=x1b[:],
                                 s0=float(c["ncp"]), s1=float(c["ir"]),
                                 imm2=float(c["dm1"]))
                            cdve(ops["cc4"], out=cc4[:], in0=q[:], in1=t2q[:],
                                 s0=0.25, s1=float(c["c01x4"]))
                            cdve(ops["z"], out=nz2, in0=nz2, in1=cc4[:],
                                 s0=float(c["na"]), s1=0.25)
                            zprev = nz2
                        else:
                            cdve(ops["x1sat"], out=x1b[:], in0=x1b[:],
                                 s0=float(c["ncp"]), s1=float(c["iff"]),
                                 imm2=float(c["aa_sat"]))
                            cdve(ops["pzsat"], out=nz2, in0=x1b[:], in1=nz2,
                                 s0=float(c["cA"]), s1=float(c["cB"]))
                    nc.sync.dma_start(out=zoh[:, t0:t1], in_=nzt_[:])

                nc.sync.dma_start(out=sto[0], in_=G[:])
                nc.sync.dma_start(out=sto[1], in_=x1b[:])

            if reps == 1:
                body()
            else:
                with tc.For_i(0, reps, 1):
                    body()

    _fix_sync_waits(nc)
    # Populate .instr bytes for InstISA subclasses (custom DVE ops); the
    # NEFF compiler rejects empty .instr with "ISA wrong length".
    mybir.codegen_inst_isa_subclasses(nc)
    return nc


# ------------------------------------------------------------ exact fallback
def _numpy_exact(zn, nz, scal):
    """Vectorized exact fp32 simulation of the reference recurrence."""
    cb, iff, ir, cp, dm, na = (f32(scal[k]) for k in (
        "coupling_base", "internal_forward", "internal_reverse",
        "center_pull", "damping", "noise_amplitude"))
    dm1 = f32(f32(1.0) - dm)
    Tn, Nn = zn.shape
    out = np.empty((Tn, Nn), f32)
    Z = np.zeros(Nn, f32)
    X1 = np.zeros(Nn, f32)
    E = np.full(Nn, 0.01, f32)
    with np.errstate(all="ignore"):
        for t in range(Tn):
            ten = np.abs(f32(zn[t] - Z))
            E = np.clip(f32(f32(E * f32(0.98)) + f32(ten * f32(100.0))),
                        0.0, 1e6).astype(f32)
            coup = np.where(ten < f32(0.01), f32(-cb), cb).astype(f32)
            X1 = f32(f32(X1 + iff) + f32(E * f32(0.005)))
            X1 = f32(X1 - f32(cp * X1))
            phi = f32(f32(0.5) * f32(X1 + f32(X1 - ir)))
            X3 = f32(phi * dm1)
            Y = np.abs(f32(X3 - phi))
            raw = f32(f32(f32(X3 * Y) + f32(coup * f32(0.1))) + f32(na * nz[t]))
            Z = np.where(np.isfinite(raw), raw, f32(0.0)).astype(f32)
            out[t] = Z
    return out


# ---------------------------------------------------------------- driver
_nc_cache = {}


def _prep_core_inputs(zn, nz, cc):
    ins = []
    for ci in range(NCORES):
        sl = slice(ci * NL, (ci + 1) * NL)
        znh = np.ascontiguousarray(
            zn[:TC1, sl].reshape(TC1, P, FDN).transpose(1, 0, 2))
        nzh = np.ascontiguousarray(
            nz[:TC2, sl].reshape(TC2, P, FDN).transpose(1, 0, 2))
        nztc = np.ascontiguousarray(nz[TC2:, sl])
        ins.append({"znh": znh, "nzh": nzh, "nzt": nztc,
                    "cc": cc.reshape(TTAIL, 1)})
    return ins


def kernel(**inputs):
    zn = np.ascontiguousarray(np.asarray(inputs["z_neighbors"], dtype=f32))
    nz = np.ascontiguousarray(np.asarray(inputs["noise"], dtype=f32))
    scal = {k: f32(inputs[k]) for k in (
        "coupling_base", "internal_forward", "internal_reverse",
        "center_pull", "damping", "noise_amplitude")}
    assert zn.shape == (T, N) and nz.shape == (T, N)
    c = _consts(scal)

    if not c["x1_fused_ok"]:
        return _numpy_exact(zn, nz, scal)

    key = tuple(float(scal[k]) for k in sorted(scal))
    if ("main", key) not in _nc_cache:
        _nc_cache[("main", key)] = _build_main(scal)
    nc = _nc_cache[("main", key)]

    xa = _attractor(c)
    cc = _cc_orbit(c, xa, TTAIL)
    in_maps = _prep_core_inputs(zn, nz, cc)
    res = run_bass_kernel_spmd(nc, in_maps, core_ids=list(range(NCORES))).results

    out = np.empty((T, N), f32)
    ok = np.isfinite(cc).all()
    for ci in range(NCORES):
        sl = slice(ci * NL, (ci + 1) * NL)
        out[:TC2, sl] = (res[ci]["zout_h"].transpose(1, 0, 2)
                         .reshape(TC2, NL))
        out[TC2:, sl] = res[ci]["zout_t"]
        st = res[ci]["state_out"]
        E_end = st[0] * f32(4.0)                      # G -> E (exact scaling)
        x1b_end = st[1]
        X1_191 = f32(f32(x1b_end * c["ncp"]) + x1b_end)
        if not np.all(E_end == SAT_E):
            ok = False
        if not np.all(X1_191 == xa):
            ok = False

    if ok:
        # E stays clipped and coupling stays positive for t>=TC1 iff
        # tension = |zn_t - Z_{t-1}| >= 200.01; Z is huge, zn is tiny.
        zmin = float(np.abs(out[TC1 - 1:]).min())
        znmax = float(np.abs(zn[TC1:]).max())
        if not np.isfinite(out[TC1:]).all() or zmin - znmax < 300.0:
            ok = False

    if not ok:
        return _numpy_exact(zn, nz, scal)
    return out


if __name__ == "__main__":
    rng = np.random.default_rng(0)
    demo = {
        "z_neighbors": rng.standard_normal((T, N), dtype=np.float32) * 0.1,
        "noise": rng.standard_normal((T, N), dtype=np.float32),
        "coupling_base": np.float32(0.05),
        "internal_forward": np.float32(0.02),
        "internal_reverse": np.float32(0.01),
        "center_pull": np.float32(0.3),
        "damping": np.float32(0.01),
        "noise_amplitude": np.float32(0.001),
    }
    o = kernel(**demo)
    print("kernel ran:", o.shape, o.dtype, float(np.abs(o).max()))
